# revision 18
# baseline (speedup 1.0000x reference)
"""Trainium2 Bass kernel for nn_ConstGCN.

Math note: in the reference, the attention score s[b,i] is constant along
the softmax axis j, and softmax is shift-invariant, so
p = softmax(s + mask) = softmax(mask) and p.sum(axis=2) == 1 (to ~1e-6 in
f32).  The output therefore collapses to

    out = relu(text + mean_k(emb_table[const_labels[...,k]]) @ fc_W.T + fc_b)

which depends on neither const_mat nor attn_W/attn_b.  The embedding + fc
further fuse into a single gather table M2 = (emb_table @ fc_W.T)/8, so

    out[b,l,:] = relu(text[b,l,:] + sum_k M2[labels[b,l,k], :] + fc_b)

On device (per core, data-parallel over batch: 2 of 16 batches = 4096
positions), per 512-position chunk:
  - DVE builds one-hot matches over the 100 label classes (is_equal vs a
    replicated iota constant, 2x bf16 mode) and sums over K=8 with a
    three-level TT-add tree.  All elementwise stays on DVE: GpSimd TT is
    2x slower AND its SBUF-port contention slows concurrent DVE ops by
    ~40% (HW-measured), ACT has no tensor_tensor.
  - text streams in via SWDGE DMA with an inline f32 -> bf16 cast (GpSimd
    only generates descriptors; no engine compute)
  - PE transposes counts to [class, position] via identity matmul; class
    row 100 (count 1, M2 row 100 = fc_b) folds the bias into the matmul
  - PE matmul counts.T @ M2 (bf16, f32 accumulate) -> PSUM, then one
    512-col bf16 identity matmul streams text into the same PSUM bank --
    the text add costs no DVE/ACT/DMA-accum time.  (The earlier float32r
    variant was abandoned: f32r lowers to fp32_mode=HIGH at ~2.2 cyc/col
    with a 300ns fp32 LDWEIGHTS, and two adjacent f32r matmuls corrupt
    the first one's output on HW.)
  - ACT does relu fused into the PSUM -> SBUF copy; HWDGE DMA on the
    scalar ring writes out
const_mat (256 MiB) is never read.
"""

import numpy as np
import ml_dtypes

B, L, D = 16, 2048, 256
CN, K = 100, 8
NCLS = 128         # 100 label classes + bias class 100 (M2 row 100 = fc_b,
                   # rows 101..127 zero; count rows 100..127 are constant 1)
NCORES = 8
POS = (B // NCORES) * L          # 4096 positions per core
CHUNK = 512                      # positions per chunk
NCHUNK = POS // CHUNK            # 8
Q = CHUNK // 128                 # 4 position-groups of 128 per chunk

_compiled = None


def _build():
    import concourse.bacc as bacc
    import concourse.mybir as mybir
    from concourse.tile import TileContext

    f32 = mybir.dt.float32
    bf16 = mybir.dt.bfloat16

    nc = bacc.Bacc("TRN2", target_bir_lowering=False)

    text_d = nc.dram_tensor("text", [POS, D], f32, kind="ExternalInput")
    lab_d = nc.dram_tensor("labels", [NCHUNK, 128, Q * K], bf16,
                           kind="ExternalInput")
    m2_d = nc.dram_tensor("m2", [NCLS, D], bf16, kind="ExternalInput")
    out_d = nc.dram_tensor("out", [POS, D], f32, kind="ExternalOutput")

    # constants embedded in the NEFF
    iota_np = np.repeat(np.arange(CN, dtype=np.float32), K)  # [CN*K]
    iota_np = np.broadcast_to(iota_np, (128, CN * K)).astype(ml_dtypes.bfloat16)
    iota_d = nc.inline_tensor(np.ascontiguousarray(iota_np), name="iota")
    ident_d = nc.inline_tensor(np.eye(128, dtype=ml_dtypes.bfloat16), name="ident")

    # position index within a chunk: partition p, group q  <->  p*Q + q
    text_v = text_d.rearrange("(n p q) d -> n p (q d)", p=128, q=Q)
    out_v = out_d.rearrange("(n p q) d -> n p (q d)", p=128, q=Q)

    with TileContext(nc) as tc:
        with (
            tc.tile_pool(name="const", bufs=1) as cpool,
            tc.tile_pool(name="work", bufs=2) as wpool,
            tc.tile_pool(name="io", bufs=3) as iopool,
            tc.tile_pool(name="ps_t", bufs=2, space="PSUM") as pst,
            tc.tile_pool(name="ps_a", bufs=6, space="PSUM") as psa,
        ):
            # first chunk's dependencies first so DVE starts ASAP
            iota_sb = cpool.tile([128, CN * K], bf16)
            nc.sync.dma_start(out=iota_sb[:, :], in_=iota_d[:, :])
            lab_tiles = []
            for n in range(NCHUNK):
                lab = wpool.tile([128, Q * K], bf16, tag="lab")
                nc.sync.dma_start(out=lab[:, :], in_=lab_d[n, :, :])
                lab_tiles.append(lab)
                if n == 0:
                    ident_sb = cpool.tile([128, 128], bf16)
                    nc.sync.dma_start(out=ident_sb[:, :], in_=ident_d[:, :])
                    m2_sb = cpool.tile([NCLS, D], bf16)
                    nc.sync.dma_start(out=m2_sb[:, :], in_=m2_d[:, :])

            # persistent counts.T tiles, double-buffered per h; rows 96..127
            # are set to one exactly once -- the per-chunk copy only
            # overwrites 0..99, so row 100 stays 1 and feeds the fc_b row
            ct_tiles = []
            for i in range(4):
                ct = cpool.tile([NCLS, 256], bf16, tag=f"ct{i}")
                nc.gpsimd.memset(ct[96:128, :], 1.0)
                ct_tiles.append(ct)

            for n in range(NCHUNK):
                lab = lab_tiles[n]

                # SWDGE cast-DMA: f32 in HBM -> bf16 in SBUF
                text_sb = iopool.tile([128, Q * D], bf16, tag="text")
                nc.gpsimd.dma_start(out=text_sb[:, :], in_=text_v[n, :, :])

                eq = wpool.tile([128, Q * CN * K], bf16, tag="eq")
                eq3 = eq.rearrange("p (q c k) -> p q c k", c=CN, k=K)
                nc.vector.tensor_tensor(
                    out=eq3,
                    in0=lab.rearrange("p (q k) -> p q k", k=K)[:, :, None, :]
                        .broadcast_to([128, Q, CN, K]),
                    in1=iota_sb.rearrange("p (c k) -> p c k", k=K)[:, None, :, :]
                        .broadcast_to([128, Q, CN, K]),
                    op=mybir.AluOpType.is_equal,
                )

                # sum over k via a TT-add tree (all DVE, 2x bf16 mode)
                s1 = wpool.tile([128, Q * CN * 4], bf16, tag="s1")
                s13 = s1.rearrange("p (q c k) -> p q c k", c=CN, k=4)
                nc.vector.tensor_add(out=s13, in0=eq3[:, :, :, 0:4],
                                     in1=eq3[:, :, :, 4:8])
                s2 = wpool.tile([128, Q * CN * 2], bf16, tag="s2")
                s23 = s2.rearrange("p (q c k) -> p q c k", c=CN, k=2)
                nc.vector.tensor_add(out=s23, in0=s13[:, :, :, 0:2],
                                     in1=s13[:, :, :, 2:4])
                counts = wpool.tile([128, Q * CN], bf16, tag="counts")
                nc.vector.tensor_add(
                    out=counts.rearrange("p (q c) -> p q c", c=CN),
                    in0=s23[:, :, :, 0],
                    in1=s23[:, :, :, 1],
                )

                res = iopool.tile([128, Q * D], f32, tag="res")
                for h in range(Q // 2):
                    ctp = pst.tile([CN, 256], bf16, tag="ctp")
                    for j in range(2):
                        q = 2 * h + j
                        nc.tensor.transpose(
                            out=ctp[:, j * 128:(j + 1) * 128],
                            in_=counts[:, q * CN:(q + 1) * CN],
                            identity=ident_sb[:, :],
                        )
                    ct = ct_tiles[(n % 2) * 2 + h]
                    nc.scalar.copy(out=ct[:CN, :], in_=ctp[:, :])

                    acc = psa.tile([128, 2 * D], f32, tag="acc")
                    # start=True resets has_written for the WHOLE PSUM bank
                    # (HW-measured), so each region's start->stop chain must
                    # complete before the next region's start in this bank
                    for j in range(2):
                        q = 2 * h + j
                        nc.tensor.matmul(
                            acc[:, j * D:(j + 1) * D],
                            lhsT=ct[:, j * 128:(j + 1) * 128],
                            rhs=m2_sb[:, :],
                            start=True, stop=False,
                        )
                        nc.tensor.matmul(
                            acc[:, j * D:(j + 1) * D],
                            lhsT=ident_sb[:, :],
                            rhs=text_sb[:, q * D:(q + 1) * D],
                            start=False, stop=True,
                        )
                    nc.scalar.activation(
                        out=res[:, h * 2 * D:(h + 1) * 2 * D],
                        in_=acc[:, :],
                        func=mybir.ActivationFunctionType.Relu,
                    )

                nc.scalar.dma_start(out=out_v[n, :, :], in_=res[:, :])

    nc.finalize()
    return nc


def _get_compiled():
    global _compiled
    if _compiled is None:
        _compiled = _build()
    return _compiled


def _prep_core_inputs(text, labels_bf16, m2):
    """text: [POS, D] f32, labels_bf16: [POS, K] bf16 -> in_map."""
    lab = labels_bf16.reshape(NCHUNK, 128, Q, K)  # (n, p, q, k): pos = n*CHUNK + p*Q + q
    lab = np.ascontiguousarray(lab.reshape(NCHUNK, 128, Q * K))
    return {
        "text": np.ascontiguousarray(text),
        "labels": lab,
        "m2": m2,
    }


def kernel(text, const_mat, const_labels, emb_table, attn_W, attn_b,
           fc_W, fc_b):
    from concourse.bass_utils import run_bass_kernel_spmd

    text = np.asarray(text, dtype=np.float32)
    const_labels = np.asarray(const_labels)
    emb_table = np.asarray(emb_table, dtype=np.float32)
    fc_W = np.asarray(fc_W, dtype=np.float32)
    fc_b = np.asarray(fc_b, dtype=np.float32)

    # fused gather table: row c (c<CN) = (emb_table @ fc_W.T)[c]/8,
    # row 100 = fc_b (count rows 100..127 are constant 1; 101..127 are 0)
    m2 = np.zeros((NCLS, D), dtype=np.float64)
    m2[:CN] = emb_table.astype(np.float64) @ fc_W.T.astype(np.float64) * 0.125
    m2[CN] = fc_b
    m2 = m2.astype(ml_dtypes.bfloat16)

    lab_bf16 = const_labels.reshape(B * L, K).astype(ml_dtypes.bfloat16)
    text_flat = text.reshape(B * L, D)

    nc = _get_compiled()
    in_maps = []
    for c in range(NCORES):
        sl = slice(c * POS, (c + 1) * POS)
        in_maps.append(_prep_core_inputs(text_flat[sl], lab_bf16[sl], m2))

    r = run_bass_kernel_spmd(nc, in_maps, core_ids=list(range(NCORES)))
    out = np.concatenate([r.results[c]["out"] for c in range(NCORES)], axis=0)
    return out.reshape(B, L, D)


# revision 19
# speedup vs baseline: 1.1281x; 1.1281x over previous
"""Trainium2 Bass kernel for nn_ConstGCN.

Math note: in the reference, the attention score s[b,i] is constant along
the softmax axis j, and softmax is shift-invariant, so
p = softmax(s + mask) = softmax(mask) and p.sum(axis=2) == 1 (to ~1e-6 in
f32).  The output therefore collapses to

    out = relu(text + mean_k(emb_table[const_labels[...,k]]) @ fc_W.T + fc_b)

which depends on neither const_mat nor attn_W/attn_b.  The embedding + fc
further fuse into a single gather table M2 = (emb_table @ fc_W.T)/8, so

    out[b,l,:] = relu(text[b,l,:] + sum_k M2[labels[b,l,k], :] + fc_b)

On device (per core, data-parallel over batch: 2 of 16 batches = 4096
positions), per 1024-position chunk:
  - DVE builds one-hot matches over the 100 label classes (is_equal vs a
    replicated iota constant, 2x bf16 mode) and sums over K=8 with a
    three-level TT-add tree.  All elementwise stays on DVE: GpSimd TT is
    2x slower AND its SBUF-port contention slows concurrent DVE ops by
    ~40% (HW-measured), ACT has no tensor_tensor.
  - text is cast to bf16 on the host (input-dtype prep, like the label
    cast) -- halves text HBM traffic and removes all on-device casts
  - iota/identity/M2/labels are packed into ONE [128, 1440] bf16 host
    array loaded by a single DMA: many small early DMAs ran at 43 B/ns
    and stalled the first chunk by ~4us (HW-measured)
  - PE transposes counts to [class, position] via identity matmul; class
    row 100 (count 1, M2 row 100 = fc_b) folds the bias into the matmul
  - per PSUM bank: ONE 512-col bf16 identity matmul writes text raw with
    start=True (sets has_written for the whole bank), then the two
    counts.T @ M2 matmuls accumulate with start=False.  start=True
    resets has_written for the WHOLE bank (HW-measured), so interleaved
    accumulation chains in one bank are forbidden -- writing text first
    sidesteps that and saves an identity reload.
  - ACT does relu fused into the PSUM -> SBUF copy; HWDGE DMA on the
    scalar ring writes out
const_mat (256 MiB) is never read.
"""

import numpy as np
import ml_dtypes

B, L, D = 16, 2048, 256
CN, K = 100, 8
NCLS = 128         # 100 label classes + bias class 100 (M2 row 100 = fc_b,
                   # rows 101..127 zero; count rows 100..127 are constant 1)
NCORES = 8
POS = (B // NCORES) * L          # 4096 positions per core
CHUNK = 1024                     # positions per chunk
NCHUNK = POS // CHUNK            # 4
Q = CHUNK // 128                 # 8 position-groups of 128 per chunk

# packed const layout (bf16 columns)
_IOTA0 = 0
_IDENT0 = _IOTA0 + CN * K        # 800
_M20 = _IDENT0 + 128             # 928
_LAB0 = _M20 + D                 # 1184
_CW = _LAB0 + NCHUNK * Q * K     # 1440

_compiled = None


def _build():
    import concourse.bacc as bacc
    import concourse.mybir as mybir
    from concourse.tile import TileContext

    f32 = mybir.dt.float32
    bf16 = mybir.dt.bfloat16

    nc = bacc.Bacc("TRN2", target_bir_lowering=False)

    consts_d = nc.dram_tensor("consts", [128, _CW], bf16, kind="ExternalInput")
    text_d = nc.dram_tensor("text", [POS, D], bf16, kind="ExternalInput")
    out_d = nc.dram_tensor("out", [POS, D], f32, kind="ExternalOutput")

    # position index within a chunk: partition p, group q  <->  p*Q + q
    text_v = text_d.rearrange("(n p q) d -> n p (q d)", p=128, q=Q)
    out_v = out_d.rearrange("(n p q) d -> n p (q d)", p=128, q=Q)

    with TileContext(nc) as tc:
        with (
            tc.tile_pool(name="const", bufs=1) as cpool,
            tc.tile_pool(name="work", bufs=2) as wpool,
            tc.tile_pool(name="io", bufs=2) as iopool,
            tc.tile_pool(name="ps_t", bufs=2, space="PSUM") as pst,
            tc.tile_pool(name="ps_a", bufs=6, space="PSUM") as psa,
        ):
            consts_sb = cpool.tile([128, _CW], bf16)
            nc.sync.dma_start(out=consts_sb[:, :], in_=consts_d[:, :])
            iota_sb = consts_sb[:, _IOTA0:_IOTA0 + CN * K]
            ident_sb = consts_sb[:, _IDENT0:_IDENT0 + 128]
            m2_sb = consts_sb[:, _M20:_M20 + D]

            # persistent counts.T tiles, double-buffered per h; rows 96..127
            # are set to one exactly once -- the per-chunk copy only
            # overwrites 0..99, so row 100 stays 1 and feeds the fc_b row
            ct_tiles = []
            for i in range(2 * (Q // 2)):
                ct = cpool.tile([NCLS, 256], bf16, tag=f"ct{i}")
                nc.gpsimd.memset(ct[96:128, :], 1.0)
                ct_tiles.append(ct)

            for n in range(NCHUNK):
                lab = consts_sb[:, _LAB0 + n * Q * K:_LAB0 + (n + 1) * Q * K]

                text_sb = iopool.tile([128, Q * D], bf16, tag="text")
                nc.sync.dma_start(out=text_sb[:, :], in_=text_v[n, :, :])

                eq = wpool.tile([128, Q * CN * K], bf16, tag="eq")
                eq3 = eq.rearrange("p (q c k) -> p q c k", c=CN, k=K)
                nc.vector.tensor_tensor(
                    out=eq3,
                    in0=lab.rearrange("p (q k) -> p q k", k=K)[:, :, None, :]
                        .broadcast_to([128, Q, CN, K]),
                    in1=iota_sb.rearrange("p (c k) -> p c k", k=K)[:, None, :, :]
                        .broadcast_to([128, Q, CN, K]),
                    op=mybir.AluOpType.is_equal,
                )

                # sum over k via a TT-add tree (all DVE, 2x bf16 mode)
                s1 = wpool.tile([128, Q * CN * 4], bf16, tag="s1")
                s13 = s1.rearrange("p (q c k) -> p q c k", c=CN, k=4)
                nc.vector.tensor_add(out=s13, in0=eq3[:, :, :, 0:4],
                                     in1=eq3[:, :, :, 4:8])
                s2 = wpool.tile([128, Q * CN * 2], bf16, tag="s2")
                s23 = s2.rearrange("p (q c k) -> p q c k", c=CN, k=2)
                nc.vector.tensor_add(out=s23, in0=s13[:, :, :, 0:2],
                                     in1=s13[:, :, :, 2:4])
                counts = wpool.tile([128, Q * CN], bf16, tag="counts")
                nc.vector.tensor_add(
                    out=counts.rearrange("p (q c) -> p q c", c=CN),
                    in0=s23[:, :, :, 0],
                    in1=s23[:, :, :, 1],
                )

                res = iopool.tile([128, Q * D], f32, tag="res")
                for h in range(Q // 2):
                    ctp = pst.tile([CN, 256], bf16, tag="ctp")
                    for j in range(2):
                        q = 2 * h + j
                        nc.tensor.transpose(
                            out=ctp[:, j * 128:(j + 1) * 128],
                            in_=counts[:, q * CN:(q + 1) * CN],
                            identity=ident_sb,
                        )
                    ct = ct_tiles[(n % 2) * (Q // 2) + h]
                    nc.scalar.copy(out=ct[:CN, :], in_=ctp[:, :])

                    acc = psa.tile([128, 2 * D], f32, tag="acc")
                    # text first with start=True: raw-writes the whole bank
                    # and sets has_written, so both ct matmuls can then
                    # accumulate with start=False.  (start=True resets
                    # has_written for the WHOLE bank, so interleaved
                    # region-wise start/stop chains corrupt the bank.)
                    nc.tensor.matmul(
                        acc[:, :],
                        lhsT=ident_sb,
                        rhs=text_sb[:, h * 2 * D:(h + 1) * 2 * D],
                        start=True, stop=False,
                        skip_group_check=True,
                    )
                    for j in range(2):
                        nc.tensor.matmul(
                            acc[:, j * D:(j + 1) * D],
                            lhsT=ct[:, j * 128:(j + 1) * 128],
                            rhs=m2_sb,
                            start=False, stop=True,
                            skip_group_check=True,
                        )
                    nc.scalar.activation(
                        out=res[:, h * 2 * D:(h + 1) * 2 * D],
                        in_=acc[:, :],
                        func=mybir.ActivationFunctionType.Relu,
                    )

                nc.scalar.dma_start(out=out_v[n, :, :], in_=res[:, :])

    nc.finalize()
    return nc


def _get_compiled():
    global _compiled
    if _compiled is None:
        _compiled = _build()
    return _compiled


def _pack_consts(m2, labels_bf16):
    """m2: [NCLS, D] bf16; labels_bf16: [POS, K] bf16 -> [128, _CW] bf16."""
    c = np.zeros((128, _CW), dtype=ml_dtypes.bfloat16)
    iota = np.repeat(np.arange(CN, dtype=np.float32), K)
    c[:, _IOTA0:_IDENT0] = iota.astype(ml_dtypes.bfloat16)[None, :]
    c[:, _IDENT0:_M20] = np.eye(128, dtype=ml_dtypes.bfloat16)
    c[:, _M20:_LAB0] = m2
    # (n, p, q, k): pos = n*CHUNK + p*Q + q
    lab = labels_bf16.reshape(NCHUNK, 128, Q * K)
    c[:, _LAB0:] = np.transpose(lab, (1, 0, 2)).reshape(128, NCHUNK * Q * K)
    return np.ascontiguousarray(c)


def kernel(text, const_mat, const_labels, emb_table, attn_W, attn_b,
           fc_W, fc_b):
    from concourse.bass_utils import run_bass_kernel_spmd

    text = np.asarray(text, dtype=np.float32)
    const_labels = np.asarray(const_labels)
    emb_table = np.asarray(emb_table, dtype=np.float32)
    fc_W = np.asarray(fc_W, dtype=np.float32)
    fc_b = np.asarray(fc_b, dtype=np.float32)

    # fused gather table: row c (c<CN) = (emb_table @ fc_W.T)[c]/8,
    # row 100 = fc_b (count rows 100..127 are constant 1; 101..127 are 0)
    m2 = np.zeros((NCLS, D), dtype=np.float64)
    m2[:CN] = emb_table.astype(np.float64) @ fc_W.T.astype(np.float64) * 0.125
    m2[CN] = fc_b
    m2 = m2.astype(ml_dtypes.bfloat16)

    lab_bf16 = const_labels.reshape(B * L, K).astype(ml_dtypes.bfloat16)
    text_bf16 = text.reshape(B * L, D).astype(ml_dtypes.bfloat16)

    nc = _get_compiled()
    in_maps = []
    for c in range(NCORES):
        sl = slice(c * POS, (c + 1) * POS)
        in_maps.append({
            "consts": _pack_consts(m2, lab_bf16[sl]),
            "text": np.ascontiguousarray(text_bf16[sl]),
        })

    r = run_bass_kernel_spmd(nc, in_maps, core_ids=list(range(NCORES)))
    out = np.concatenate([r.results[c]["out"] for c in range(NCORES)], axis=0)
    return out.reshape(B, L, D)
